# revision 5
# baseline (speedup 1.0000x reference)
"""Trainium2 Bass kernel for BitConv2d:
GroupNorm(8) -> ReLU^2 -> PACT 8-bit quant -> 3x3 conv (ternary weight) -> bias.

Strategy (data-parallel over batch, 8 cores x 4 images):
 - Host: ternarize the [256,256,3,3] weight (exact forward value is
   alpha_oc * sign(w) * mask); keep only the {-1,0,+1} pattern for the device
   (fp16), folding alpha_oc/S into a per-out-channel rescale applied at PSUM
   evacuation time.
 - Device per image: GroupNorm stats via bn_stats (per-partition mean/var)
   + two tiny PE matmuls (group-reduce over 32-channel groups and broadcast
   back, with gamma folded into the broadcast matrix).  Then one ACT pass
   relu(A*x+B), one ACT pass Square(sqrt(S)*u) and two DVE passes that clip
   to 255, round to the integer quantization grid exactly (fp32 +1.5*2^23
   trick) and write n+1024 into a zero^H^H 1024-padded fp16 [128,58,58]
   image.  Because the padding is also 1024, conv(n+1024) differs from
   conv(n) by a per-out-channel constant that is folded into the output bias.
 - Conv: out[co, 8x56 spatial block] accumulates 18 matmuls
   (9 taps x 2 cin chunks) of [128k x 128m] @ [128k x 448n] into PSUM;
   evacuation = ACT Identity(psum * scale_co + bias'_co) then DMA out.
All quantities n+1024 in [1024,1279] are exactly representable in fp16 and
the fp32 PSUM accumulation of <= 2304 such integer terms is exact, so the
conv is bit-exact integer arithmetic.
"""

import os
import sys

import numpy as np

for _p in ("/opt/trn_rl_repo", "/root/.axon_site/_ro/trn_rl_repo"):
    if _p not in sys.path and os.path.isdir(_p):
        sys.path.append(_p)

GN_GROUPS = 8
GN_EPS = 1e-5
K_BITS = 8
DELTA = 0.05
EPS = 1e-8

B_TOT, C, H, W = 32, 256, 56, 56
HW = H * W  # 3136
PW = W + 2  # 58
N_CORES = 8
B_LOC = B_TOT // N_CORES  # 4
RB = 7          # row blocks per image
RBH = H // RB   # 8 rows per block
NN = RBH * W    # 448 columns per conv matmul
C32 = 12582912.0  # 1.5 * 2**23, fp32 round-to-nearest-int trick
OFF = 1024.0      # fp16 integer-exact offset


def _host_prep(gamma, beta, a, weight_fp, bias):
    """Ternarize weights and pack all small device constants (numpy)."""
    w = weight_fp.astype(np.float32)
    wv = w.reshape(C, -1)
    wa = np.abs(wv)
    t = (DELTA * wa.mean(axis=1, keepdims=True)).astype(np.float32)
    m = (wa > t)
    alpha = ((wa * m).sum(axis=1, dtype=np.float64)
             / (m.sum(axis=1).astype(np.float64) + EPS)).astype(np.float32)
    tern = (np.sign(wv) * m).astype(np.float32)  # [-1, 0, 1]
    t_co = tern.sum(axis=1)  # integer-valued, per out channel

    a_c = np.float32(max(float(a), 0.0) + EPS)
    S = np.float32((2.0 ** K_BITS - 1.0) / a_c)
    sqrt_s = float(np.float32(np.sqrt(np.float64(S))))

    # out = (alpha_co / S) * conv(n, tern) + bias_co
    # psum = conv(n + OFF, tern) = conv(n, tern) + OFF * t_co
    scale = (alpha / S).astype(np.float32)
    bias2 = (bias.astype(np.float64) - np.float64(OFF) * scale.astype(np.float64)
             * t_co.astype(np.float64)).astype(np.float32)

    # lhsT layout: wt[p, c, k, q, m] = tern[oc=q*128+m, ci=c*128+p, k]
    tern4 = tern.reshape(2, 128, 2, 128, 9)          # [q, m, c, p, k]
    wt = np.ascontiguousarray(tern4.transpose(3, 2, 4, 0, 1)).astype(np.float16)

    sc = np.empty((2, 128, 2), np.float32)
    sc[:, :, 0] = scale.reshape(2, 128)
    sc[:, :, 1] = bias2.reshape(2, 128)

    gb = np.empty((2, 128, 2), np.float32)
    gb[:, :, 0] = gamma.astype(np.float32).reshape(2, 128)
    gb[:, :, 1] = beta.astype(np.float32).reshape(2, 128)

    ind = np.zeros((128, 4), np.float32)
    ind[np.arange(128), np.arange(128) // 32] = 1.0 / 32.0

    indtg = np.zeros((2, 4, 128), np.float32)
    for cch in range(2):
        for p in range(128):
            indtg[cch, p // 32, p] = gamma.astype(np.float32)[cch * 128 + p]

    return dict(wt=wt, sc=sc, gb=gb, ind=ind, indtg=indtg, sqrt_s=sqrt_s)


def _build_nc(sqrt_s):
    import concourse.bacc as bacc
    import concourse.mybir as mybir
    import concourse.tile as tile
    from contextlib import ExitStack

    f32 = mybir.dt.float32
    f16 = mybir.dt.float16
    AF = mybir.ActivationFunctionType
    ALU = mybir.AluOpType

    # Bacc (not raw Bass): its compile() runs generate_event_semaphores,
    # which legalizes multi-sem waits down to the 1-wait-per-instruction
    # TRN2 ISA constraint.
    nc = bacc.Bacc("TRN2", target_bir_lowering=False, debug=False,
                   num_devices=N_CORES)

    x_ap = nc.dram_tensor("x", [B_LOC, C, HW], f32, kind="ExternalInput").ap()
    wt_ap = nc.dram_tensor("wt", [128, 2, 9, 2, 128], f16,
                           kind="ExternalInput").ap()
    sc_ap = nc.dram_tensor("sc", [2, 128, 2], f32, kind="ExternalInput").ap()
    gb_ap = nc.dram_tensor("gb", [2, 128, 2], f32, kind="ExternalInput").ap()
    ind_ap = nc.dram_tensor("ind", [128, 4], f32, kind="ExternalInput").ap()
    indtg_ap = nc.dram_tensor("indtg", [2, 4, 128], f32,
                              kind="ExternalInput").ap()
    out_ap = nc.dram_tensor("out", [B_LOC, C, HW], f32,
                            kind="ExternalOutput").ap()

    with tile.TileContext(nc) as tc, ExitStack() as ctx:
        consts = ctx.enter_context(tc.tile_pool(name="consts", bufs=1))
        x_pool = ctx.enter_context(tc.tile_pool(name="xp", bufs=2))
        u_pool = ctx.enter_context(tc.tile_pool(name="up", bufs=2))
        pad_pool = ctx.enter_context(tc.tile_pool(name="padp", bufs=1))
        st_pool = ctx.enter_context(tc.tile_pool(name="stp", bufs=2))
        tiny = ctx.enter_context(tc.tile_pool(name="tinyp", bufs=4))
        out_pool = ctx.enter_context(tc.tile_pool(name="outp", bufs=6))
        cps_pool = ctx.enter_context(tc.tile_pool(name="cps", bufs=3,
                                                  space="PSUM"))
        gps_pool = ctx.enter_context(tc.tile_pool(name="gps", bufs=2,
                                                  space="PSUM"))

        w_sb = consts.tile([128, 2, 9, 2, 128], f16, name="w_sb")
        nc.sync.dma_start(out=w_sb, in_=wt_ap)
        sc_sb = []
        gb_sb = []
        indtg_sb = []
        for q in range(2):
            s_t = consts.tile([128, 2], f32, name=f"sc_sb{q}", tag=f"sc{q}")
            nc.sync.dma_start(out=s_t, in_=sc_ap[q])
            sc_sb.append(s_t)
            g_t = consts.tile([128, 2], f32, name=f"gb_sb{q}", tag=f"gb{q}")
            nc.sync.dma_start(out=g_t, in_=gb_ap[q])
            gb_sb.append(g_t)
            it_t = consts.tile([4, 128], f32, name=f"indtg_sb{q}",
                               tag=f"it{q}")
            nc.sync.dma_start(out=it_t, in_=indtg_ap[q])
            indtg_sb.append(it_t)
        ind_sb = consts.tile([128, 4], f32, name="ind_sb")
        nc.sync.dma_start(out=ind_sb, in_=ind_ap)
        eps_sb = consts.tile([4, 1], f32, name="eps_sb")
        nc.vector.memset(eps_sb, GN_EPS)

        pad_t = [[pad_pool.tile([128, PW, PW], f16, name=f"pad_{cch}_{s}",
                                tag=f"pad{cch}{s}")
                  for s in range(2)] for cch in range(2)]
        for cch in range(2):
            for s in range(2):
                nc.vector.memset(pad_t[cch][s], OFF)

        xs = [[None] * 2 for _ in range(B_LOC)]
        gms = [[None] * 2 for _ in range(B_LOC)]
        abs_ = [[None] * 2 for _ in range(B_LOC)]

        def emit_load(b):
            for cch in range(2):
                xt = x_pool.tile([128, HW], f32, name=f"x_{b}_{cch}",
                                 tag=f"x{cch}")
                nc.sync.dma_start(out=xt, in_=x_ap[b, cch * 128:(cch + 1) * 128, :])
                xs[b][cch] = xt

        def emit_stats(b):
            for cch in range(2):
                st6 = st_pool.tile([128, RB, 6], f32, name=f"st6_{b}_{cch}",
                                   tag="st6")
                for j in range(RB):
                    nc.vector.bn_stats(out=st6[:, j, :],
                                       in_=xs[b][cch][:, j * NN:(j + 1) * NN])
                mv = tiny.tile([128, 2], f32, name=f"mv_{b}_{cch}", tag="mv")
                nc.vector.bn_aggr(out=mv, in_=st6)
                msq = tiny.tile([128, 1], f32, name=f"msq_{b}_{cch}", tag="msq")
                nc.vector.tensor_mul(msq, mv[:, 0:1], mv[:, 0:1])
                m2 = tiny.tile([128, 2], f32, name=f"m2_{b}_{cch}", tag="m2")
                # keep every matmul-operand producer on DVE: Matmult's
                # LDWEIGHTS slot only fits 2 sync waits (walrus NCC_INLA001)
                nc.vector.tensor_copy(out=m2[:, 0:1], in_=mv[:, 0:1])
                nc.vector.tensor_add(m2[:, 1:2], mv[:, 1:2], msq)
                gmv = gps_pool.tile([4, 2], f32, name=f"gmv_{b}_{cch}",
                                    tag="gmv")
                nc.tensor.matmul(gmv, lhsT=ind_sb, rhs=m2, start=True,
                                 stop=True)
                gm = tiny.tile([4, 2], f32, name=f"gm_{b}_{cch}", tag="gm")
                nc.vector.tensor_copy(out=gm, in_=gmv)
                gms[b][cch] = gm

        def emit_post(b):
            for cch in range(2):
                gm = gms[b][cch]
                gsq = tiny.tile([4, 1], f32, name=f"gsq_{b}_{cch}", tag="gsq")
                nc.vector.tensor_mul(gsq, gm[:, 0:1], gm[:, 0:1])
                gvar = tiny.tile([4, 1], f32, name=f"gvar_{b}_{cch}",
                                 tag="gvar")
                nc.vector.tensor_sub(gvar, gm[:, 1:2], gsq)
                # gvar <- 1/sqrt(var + eps)
                nc.scalar.activation(out=gvar, in_=gvar, func=AF.Sqrt,
                                     bias=eps_sb, scale=1.0)
                nc.vector.reciprocal(out=gvar, in_=gvar)
                vals = tiny.tile([4, 2], f32, name=f"vals_{b}_{cch}",
                                 tag="vals")
                nc.vector.tensor_mul(vals[:, 0:1], gm[:, 0:1], gvar)
                nc.vector.tensor_copy(out=vals[:, 1:2], in_=gvar)
                bc = gps_pool.tile([128, 2], f32, name=f"bc_{b}_{cch}",
                                   tag="bc")
                nc.tensor.matmul(bc, lhsT=indtg_sb[cch], rhs=vals, start=True,
                                 stop=True)
                ab = tiny.tile([128, 2], f32, name=f"ab_{b}_{cch}", tag="ab")
                nc.vector.tensor_copy(out=ab, in_=bc)
                bt = tiny.tile([128, 1], f32, name=f"bt_{b}_{cch}", tag="bt")
                nc.vector.tensor_sub(bt, gb_sb[cch][:, 1:2], ab[:, 0:1])
                abs_[b][cch] = (ab, bt)

        def emit_act(b):
            for cch in range(2):
                ab, bt = abs_[b][cch]
                u = u_pool.tile([128, HW], f32, name=f"u_{b}_{cch}",
                                tag=f"u{cch}")
                # u = relu(A*x + B)
                nc.scalar.activation(out=u, in_=xs[b][cch], func=AF.Relu,
                                     bias=bt, scale=ab[:, 1:2])
                # u = (sqrt(S)*u)^2 = S*relu(z)^2
                nc.scalar.activation(out=u, in_=u, func=AF.Square,
                                     scale=sqrt_s)
                # u = min(u, 255) + C32   (fp32 add rounds to integer, RNE)
                nc.vector.tensor_scalar(out=u, in0=u, scalar1=255.0,
                                        scalar2=C32, op0=ALU.min, op1=ALU.add)
                # pad interior = (u - C32) + 1024, exact fp16 integers
                nc.vector.tensor_scalar_add(
                    out=pad_t[cch][b % 2][:, 1:H + 1, 1:W + 1],
                    in0=u.rearrange("p (h w) -> p h w", h=H),
                    scalar1=OFF - C32)

        def emit_conv(b, q):
            for rb in range(RB):
                ps = cps_pool.tile([128, NN], f32, name=f"ps_{b}_{q}_{rb}",
                                   tag="cps")
                i = 0
                for k in range(9):
                    dy, dx = divmod(k, 3)
                    for cch in range(2):
                        nc.tensor.matmul(
                            ps,
                            lhsT=w_sb[:, cch, k, q, :],
                            rhs=pad_t[cch][b % 2][:, rb * RBH + dy:
                                                  rb * RBH + dy + RBH,
                                                  dx:dx + W],
                            start=(i == 0), stop=(i == 17))
                        i += 1
                osb = out_pool.tile([128, NN], f32, name=f"o_{b}_{q}_{rb}",
                                    tag="osb")
                nc.scalar.activation(out=osb, in_=ps, func=AF.Identity,
                                     bias=sc_sb[q][:, 1:2],
                                     scale=sc_sb[q][:, 0:1])
                nc.sync.dma_start(
                    out=out_ap[b, q * 128:(q + 1) * 128,
                               rb * NN:(rb + 1) * NN],
                    in_=osb)

        emit_load(0)
        emit_stats(0)
        emit_post(0)
        emit_act(0)
        for b in range(B_LOC):
            if b + 1 < B_LOC:
                emit_load(b + 1)
                emit_stats(b + 1)
            emit_conv(b, 0)
            if b + 1 < B_LOC:
                emit_post(b + 1)
            emit_conv(b, 1)
            if b + 1 < B_LOC:
                emit_act(b + 1)

    nc.compile()
    return nc


def kernel(x, gamma, beta, a, weight_fp, bias):
    consts = _host_prep(np.asarray(gamma), np.asarray(beta), np.asarray(a),
                        np.asarray(weight_fp), np.asarray(bias))
    nc = _build_nc(consts.pop("sqrt_s"))

    from concourse.bass_utils import run_bass_kernel_spmd

    x = np.ascontiguousarray(np.asarray(x, dtype=np.float32)
                             .reshape(B_TOT, C, HW))
    in_maps = []
    for core in range(N_CORES):
        in_maps.append({
            "x": x[core * B_LOC:(core + 1) * B_LOC],
            "wt": consts["wt"],
            "sc": consts["sc"],
            "gb": consts["gb"],
            "ind": consts["ind"],
            "indtg": consts["indtg"],
        })
    res = run_bass_kernel_spmd(nc, in_maps, list(range(N_CORES)))
    out = np.concatenate([res.results[i]["out"] for i in range(N_CORES)],
                         axis=0)
    return out.reshape(B_TOT, C, H, W)


if __name__ == "__main__":
    rng = np.random.default_rng(0)
    x = rng.standard_normal((B_TOT, C, H, W), dtype=np.float32)
    out = kernel(x, np.ones(C, np.float32), np.zeros(C, np.float32),
                 np.float32(6.0),
                 rng.standard_normal((C, C, 3, 3), dtype=np.float32) * 0.03,
                 np.zeros(C, np.float32))
    print(out.shape, out.dtype)


# revision 33
# speedup vs baseline: 361.9992x; 361.9992x over previous
"""Trainium2 Bass kernel for BitConv2d:
GroupNorm(8) -> ReLU^2 -> PACT 8-bit quant -> 3x3 conv (ternary weight) -> bias.

Strategy (data-parallel over batch, 8 cores x 4 images):
 - Host: ternarize the [256,256,3,3] weight (exact forward value is
   alpha_oc * sign(w) * mask); keep only the {-1,0,+1} pattern for the device
   (fp16), folding alpha_oc/S into a per-out-channel rescale applied at PSUM
   evacuation time.
 - Device per image: GroupNorm stats via bn_stats (per-partition mean/var)
   + two tiny PE matmuls (group-reduce over 32-channel groups and broadcast
   back, with gamma folded into the broadcast matrix).  Then one ACT pass
   relu(A*x+B), one ACT pass Square(sqrt(S)*u) and two DVE passes that clip
   to 255, round to the integer quantization grid exactly (fp32 +1.5*2^23
   trick) and write n+1024 into a zero^H^H 1024-padded fp16 [128,58,58]
   image.  Because the padding is also 1024, conv(n+1024) differs from
   conv(n) by a per-out-channel constant that is folded into the output bias.
 - Conv: out[co, 8x56 spatial block] accumulates 18 matmuls
   (9 taps x 2 cin chunks) of [128k x 128m] @ [128k x 448n] into PSUM;
   evacuation = ACT Identity(psum * scale_co + bias'_co) then DMA out.
All quantities n+1024 in [1024,1279] are exactly representable in fp16 and
the fp32 PSUM accumulation of <= 2304 such integer terms is exact, so the
conv is bit-exact integer arithmetic.
"""

import os
import sys

import numpy as np

for _p in ("/opt/trn_rl_repo", "/root/.axon_site/_ro/trn_rl_repo"):
    if _p not in sys.path and os.path.isdir(_p):
        sys.path.append(_p)

GN_GROUPS = 8
GN_EPS = 1e-5
K_BITS = 8
DELTA = 0.05
EPS = 1e-8

B_TOT, C, H, W = 32, 256, 56, 56
HW = H * W  # 3136
PW = W + 2  # 58
N_CORES = 8
B_LOC = B_TOT // N_CORES  # 4
RB = 7          # row blocks per image
RBH = H // RB   # 8 rows per block
NN = RBH * W    # 448 columns per conv matmul
C32 = 12582912.0  # 1.5 * 2**23, fp32 round-to-nearest-int trick
OFF = 1024.0      # fp16 integer-exact offset
ROUND_VIA_F16 = True  # round via fp32->fp16 RNE convert (1 DVE pass) instead
#                       of the +C32 fp32 trick (2 passes); HW-verified below


def _host_prep(gamma, beta, a, weight_fp, bias):
    """Ternarize weights and pack all small device constants (numpy)."""
    w = weight_fp.astype(np.float32)
    wv = w.reshape(C, -1)
    wa = np.abs(wv)
    t = (DELTA * wa.mean(axis=1, keepdims=True)).astype(np.float32)
    m = (wa > t)
    alpha = ((wa * m).sum(axis=1, dtype=np.float64)
             / (m.sum(axis=1).astype(np.float64) + EPS)).astype(np.float32)
    tern = (np.sign(wv) * m).astype(np.float32)  # [-1, 0, 1]
    t_co = tern.sum(axis=1)  # integer-valued, per out channel

    a_c = np.float32(max(float(a), 0.0) + EPS)
    S = np.float32((2.0 ** K_BITS - 1.0) / a_c)
    sqrt_s = float(np.float32(np.sqrt(np.float64(S))))

    # out = (alpha_co / S) * conv(n, tern) + bias_co
    # psum = conv(n + OFF, tern) = conv(n, tern) + OFF * t_co
    scale = (alpha / S).astype(np.float32)
    bias2 = (bias.astype(np.float64) - np.float64(OFF) * scale.astype(np.float64)
             * t_co.astype(np.float64)).astype(np.float32)

    # lhsT layout: wt[p, c, k, q, m] = tern[oc=q*128+m, ci=c*128+p, k]
    tern4 = tern.reshape(2, 128, 2, 128, 9)          # [q, m, c, p, k]
    wt = np.ascontiguousarray(tern4.transpose(3, 2, 4, 0, 1)).astype(np.float16)

    # one packed [128, 268] f32 constant tensor -> a single DMA:
    #   cols 0:4     ind (group-average matrix, 1/32 one-hot)
    #   cols 4:132   indtg chunk0 (rows 0..3 used: gamma-weighted one-hot.T)
    #   cols 132:260 indtg chunk1 (rows 0..3 used)
    #   cols 260:262 sc chunk0 (scale, bias')   cols 262:264 sc chunk1
    #   cols 264:266 gb chunk0 (gamma, beta)    cols 266:268 gb chunk1
    g32 = gamma.astype(np.float32)
    pack = np.zeros((128, 268), np.float32)
    pack[np.arange(128), np.arange(128) // 32] = 1.0 / 32.0
    for cch in range(2):
        for p in range(128):
            pack[p // 32, 4 + 128 * cch + p] = g32[cch * 128 + p]
        pack[:, 260 + 2 * cch] = scale.reshape(2, 128)[cch]
        pack[:, 261 + 2 * cch] = bias2.reshape(2, 128)[cch]
        pack[:, 264 + 2 * cch] = g32.reshape(2, 128)[cch]
        pack[:, 265 + 2 * cch] = beta.astype(np.float32).reshape(2, 128)[cch]

    return dict(wt=wt, pack=pack, sqrt_s=sqrt_s)


def _build_nc(sqrt_s, ablate=None):
    import concourse.bacc as bacc
    import concourse.mybir as mybir
    import concourse.tile as tile
    from contextlib import ExitStack

    f32 = mybir.dt.float32
    f16 = mybir.dt.float16
    AF = mybir.ActivationFunctionType
    ALU = mybir.AluOpType

    # Bacc (not raw Bass): its compile() runs generate_event_semaphores,
    # which legalizes multi-sem waits down to the 1-wait-per-instruction
    # TRN2 ISA constraint.
    nc = bacc.Bacc("TRN2", target_bir_lowering=False, debug=False,
                   num_devices=N_CORES)

    x_ap = nc.dram_tensor("x", [B_LOC, C, HW], f32, kind="ExternalInput").ap()
    wt_ap = nc.dram_tensor("wt", [128, 2, 9, 2, 128], f16,
                           kind="ExternalInput").ap()
    pack_ap = nc.dram_tensor("pack", [128, 268], f32,
                             kind="ExternalInput").ap()
    out_ap = nc.dram_tensor("out", [B_LOC, C, HW], f32,
                            kind="ExternalOutput").ap()

    with tile.TileContext(nc) as tc, ExitStack() as ctx:
        consts = ctx.enter_context(tc.tile_pool(name="consts", bufs=1))
        x_pool = ctx.enter_context(tc.tile_pool(name="xp", bufs=2))
        u_pool = ctx.enter_context(tc.tile_pool(name="up", bufs=2))
        pad_pool = ctx.enter_context(tc.tile_pool(name="padp", bufs=1))
        st_pool = ctx.enter_context(tc.tile_pool(name="stp", bufs=2))
        tiny = ctx.enter_context(tc.tile_pool(name="tinyp", bufs=4))
        out_pool = ctx.enter_context(tc.tile_pool(name="outp", bufs=6))
        cps_pool = ctx.enter_context(tc.tile_pool(name="cps", bufs=3,
                                                  space="PSUM"))
        gps_pool = ctx.enter_context(tc.tile_pool(name="gps", bufs=2,
                                                  space="PSUM"))

        xs = [[None] * 2 for _ in range(B_LOC)]
        gms = [[None] * 2 for _ in range(B_LOC)]
        abs_ = [[None] * 2 for _ in range(B_LOC)]

        def emit_load_chunk(b, cch, bounds=(0, HW)):
            # split loads (at bn_stats 448-block boundaries) so stats on
            # early blocks start before the tail lands (Tile subtile deps)
            xt = x_pool.tile([128, HW], f32, name=f"x_{b}_{cch}",
                             tag=f"x{cch}")
            for lo, hi in zip(bounds[:-1], bounds[1:]):
                nc.sync.dma_start(
                    out=xt[:, lo:hi],
                    in_=x_ap[b, cch * 128:(cch + 1) * 128, lo:hi])
            xs[b][cch] = xt

        def emit_load(b, bounds=(0, HW)):
            for cch in range(2):
                emit_load_chunk(b, cch, bounds)

        # image 0 is on the critical path: chunk 0's x pieces dispatch first
        # (HWDGE dispatch is ~625ns per dma_start and serializes), then the
        # packed small consts, then chunk 1, then the conv weights
        emit_load_chunk(0, 0, bounds=(0, 3 * NN, 6 * NN, HW))
        pk_sb = consts.tile([128, 268], f32, name="pk_sb")
        nc.sync.dma_start(out=pk_sb, in_=pack_ap)
        emit_load_chunk(0, 1, bounds=(0, 3 * NN, 6 * NN, HW))

        w_sb = consts.tile([128, 2, 9, 2, 128], f16, name="w_sb")
        nc.sync.dma_start(out=w_sb, in_=wt_ap)

        ind_sb = pk_sb[:, 0:4]
        indtg_sb = [pk_sb[0:4, 4:132], pk_sb[0:4, 132:260]]
        sc_sb = [pk_sb[:, 260:262], pk_sb[:, 262:264]]
        gb_sb = [pk_sb[:, 264:266], pk_sb[:, 266:268]]
        eps_sb = consts.tile([4, 1], f32, name="eps_sb")
        nc.vector.memset(eps_sb, GN_EPS)

        pad_t = [[pad_pool.tile([128, PW, PW], f16, name=f"pad_{cch}_{s}",
                                tag=f"pad{cch}{s}")
                  for s in range(2)] for cch in range(2)]
        for cch in range(2):
            for s in range(2):
                # border-only memsets (interior is overwritten every image),
                # on GPSIMD which is otherwise idle — a full-tile DVE memset
                # costs 3.5us x4 and sits right in front of bn_stats
                pt = pad_t[cch][s]
                nc.gpsimd.memset(pt[:, 0, :], OFF)
                nc.gpsimd.memset(pt[:, PW - 1, :], OFF)
                nc.gpsimd.memset(pt[:, 1:PW - 1, 0:1], OFF)
                nc.gpsimd.memset(pt[:, 1:PW - 1, PW - 1:PW], OFF)

        def emit_stats(b, chunks=(0, 1)):
            for cch in chunks:
                st6 = st_pool.tile([128, RB, 6], f32, name=f"st6_{b}_{cch}",
                                   tag="st6")
                for j in range(RB):
                    nc.vector.bn_stats(out=st6[:, j, :],
                                       in_=xs[b][cch][:, j * NN:(j + 1) * NN])
                mv = tiny.tile([128, 2], f32, name=f"mv_{b}_{cch}", tag="mv")
                nc.vector.bn_aggr(out=mv, in_=st6)
                msq = tiny.tile([128, 1], f32, name=f"msq_{b}_{cch}", tag="msq")
                nc.vector.tensor_mul(msq, mv[:, 0:1], mv[:, 0:1])
                m2 = tiny.tile([128, 2], f32, name=f"m2_{b}_{cch}", tag="m2")
                # keep every matmul-operand producer on DVE: Matmult's
                # LDWEIGHTS slot only fits 2 sync waits (walrus NCC_INLA001)
                nc.vector.tensor_copy(out=m2[:, 0:1], in_=mv[:, 0:1])
                nc.vector.tensor_add(m2[:, 1:2], mv[:, 1:2], msq)
                gmv = gps_pool.tile([4, 2], f32, name=f"gmv_{b}_{cch}",
                                    tag="gmv")
                nc.tensor.matmul(gmv, lhsT=ind_sb, rhs=m2, start=True,
                                 stop=True)
                gm = tiny.tile([4, 2], f32, name=f"gm_{b}_{cch}", tag="gm")
                nc.vector.tensor_copy(out=gm, in_=gmv)
                gms[b][cch] = gm

        def emit_post(b, chunks=(0, 1)):
            for cch in chunks:
                gm = gms[b][cch]
                gsq = tiny.tile([4, 1], f32, name=f"gsq_{b}_{cch}", tag="gsq")
                nc.vector.tensor_mul(gsq, gm[:, 0:1], gm[:, 0:1])
                gvar = tiny.tile([4, 1], f32, name=f"gvar_{b}_{cch}",
                                 tag="gvar")
                nc.vector.tensor_sub(gvar, gm[:, 1:2], gsq)
                # gvar <- 1/sqrt(var + eps)
                nc.scalar.activation(out=gvar, in_=gvar, func=AF.Sqrt,
                                     bias=eps_sb, scale=1.0)
                nc.vector.reciprocal(out=gvar, in_=gvar)
                vals = tiny.tile([4, 2], f32, name=f"vals_{b}_{cch}",
                                 tag="vals")
                nc.vector.tensor_mul(vals[:, 0:1], gm[:, 0:1], gvar)
                nc.vector.tensor_copy(out=vals[:, 1:2], in_=gvar)
                bc = gps_pool.tile([128, 2], f32, name=f"bc_{b}_{cch}",
                                   tag="bc")
                nc.tensor.matmul(bc, lhsT=indtg_sb[cch], rhs=vals, start=True,
                                 stop=True)
                ab = tiny.tile([128, 2], f32, name=f"ab_{b}_{cch}", tag="ab")
                nc.vector.tensor_copy(out=ab, in_=bc)
                bt = tiny.tile([128, 1], f32, name=f"bt_{b}_{cch}", tag="bt")
                nc.vector.tensor_sub(bt, gb_sb[cch][:, 1:2], ab[:, 0:1])
                abs_[b][cch] = (ab, bt)

        def emit_act(b, split=1, chunks=(0, 1), dve_sq=False):
            # split>1 shortens chain latency (image 0 is on the critical
            # path before any conv work exists); dve_sq moves alternate
            # quarters' square onto DVE to unbottleneck ACT at startup
            nh = split
            hrows = H // nh
            hlen = hrows * W
            for cch in chunks:
                ab, bt = abs_[b][cch]
                u = u_pool.tile([128, HW], f32, name=f"u_{b}_{cch}",
                                tag=f"u{cch}")
                for h in range(nh):
                    sl = slice(h * hlen, (h + 1) * hlen)
                    uv = u[:, sl]
                    # u = relu(A*x + B)
                    nc.scalar.activation(out=uv, in_=xs[b][cch][:, sl],
                                         func=AF.Relu,
                                         bias=bt, scale=ab[:, 1:2])
                    if dve_sq and (h + cch) % 2 == 1:
                        # u = (u*u)*S on DVE (tensor_tensor_reduce)
                        tacc = tiny.tile([128, 1], f32,
                                         name=f"tacc_{b}_{cch}_{h}",
                                         tag="tacc")
                        nc.vector.tensor_tensor_reduce(
                            out=uv, in0=uv, in1=uv, scale=sqrt_s * sqrt_s,
                            scalar=0.0, op0=ALU.mult, op1=ALU.max,
                            accum_out=tacc)
                    else:
                        # u = (sqrt(S)*u)^2 = S*relu(z)^2
                        nc.scalar.activation(out=uv, in_=uv, func=AF.Square,
                                             scale=sqrt_s)
                    if ROUND_VIA_F16:
                        # pad interior = min(u,255) + 1024; the fp32->fp16
                        # output convert rounds to the integer grid (RNE;
                        # +1024 is even so tie parity matches jnp.round)
                        nc.vector.tensor_scalar(
                            out=pad_t[cch][b % 2][:, 1 + h * hrows:
                                                  1 + (h + 1) * hrows,
                                                  1:W + 1],
                            in0=uv.rearrange("p (h w) -> p h w", h=hrows),
                            scalar1=255.0, scalar2=OFF,
                            op0=ALU.min, op1=ALU.add)
                    else:
                        # u = min(u, 255) + C32 (fp32 add rounds to int, RNE)
                        nc.vector.tensor_scalar(out=uv, in0=uv, scalar1=255.0,
                                                scalar2=C32, op0=ALU.min,
                                                op1=ALU.add)
                        # pad interior = (u - C32) + 1024, exact f16 integers
                        nc.vector.tensor_scalar_add(
                            out=pad_t[cch][b % 2][:, 1 + h * hrows:
                                                  1 + (h + 1) * hrows,
                                                  1:W + 1],
                            in0=uv.rearrange("p (h w) -> p h w", h=hrows),
                            scalar1=OFF - C32)

        def emit_conv(b, q):
            # one full-image output tile per (b, q): evacs write slices and a
            # single big DMA stores it (HWDGE dispatch is ~625ns per
            # dma_start; 7 small stores would serialize against the loads)
            osb = out_pool.tile([128, HW], f32, name=f"o_{b}_{q}", tag="osb",
                                bufs=2)
            for rb in range(RB):
                ps = cps_pool.tile([128, NN], f32, name=f"ps_{b}_{q}_{rb}",
                                   tag="cps")
                i = 0
                for cch in range(2):  # cch-outer: taps of chunk 0 can start
                    for k in range(9):  # before chunk 1's pad is written
                        dy, dx = divmod(k, 3)
                        nc.tensor.matmul(
                            ps,
                            lhsT=w_sb[:, cch, k, q, :],
                            rhs=pad_t[cch][b % 2][:, rb * RBH + dy:
                                                  rb * RBH + dy + RBH,
                                                  dx:dx + W],
                            start=(i == 0), stop=(i == 17))
                        i += 1
                nc.scalar.activation(out=osb[:, rb * NN:(rb + 1) * NN],
                                     in_=ps, func=AF.Identity,
                                     bias=sc_sb[q][:, 1:2],
                                     scale=sc_sb[q][:, 0:1])
            # piecewise stores: each piece only depends on the evacs that
            # wrote it (subtile deps).  The last image streams out
            # per-rowblock so the kernel tail isn't gated on one big DMA.
            cuts = (list(range(0, HW + 1, NN)) if b == B_LOC - 1
                    else [0, 4 * NN, HW])
            for lo, hi in zip(cuts[:-1], cuts[1:]):
                nc.sync.dma_start(out=out_ap[b, q * 128:(q + 1) * 128, lo:hi],
                                  in_=osb[:, lo:hi])

        if ablate == "conv":
            for b in range(B_LOC):
                emit_conv(b, 0)
                emit_conv(b, 1)
        elif ablate == "gn":
            emit_stats(0); emit_post(0); emit_act(0)
            for b in range(1, B_LOC):
                emit_load(b); emit_stats(b); emit_post(b); emit_act(b)
        else:
            for cch in range(2):
                emit_stats(0, chunks=(cch,))
                emit_post(0, chunks=(cch,))
                emit_act(0, split=4, chunks=(cch,))
            for b in range(B_LOC):
                if b + 1 < B_LOC:
                    emit_load(b + 1)
                    emit_stats(b + 1)
                emit_conv(b, 0)
                if b + 1 < B_LOC:
                    emit_post(b + 1)
                emit_conv(b, 1)
                if b + 1 < B_LOC:
                    emit_act(b + 1)

    nc.compile()
    return nc


def kernel(x, gamma, beta, a, weight_fp, bias):
    consts = _host_prep(np.asarray(gamma), np.asarray(beta), np.asarray(a),
                        np.asarray(weight_fp), np.asarray(bias))
    nc = _build_nc(consts.pop("sqrt_s"))

    from concourse.bass_utils import run_bass_kernel_spmd

    x = np.ascontiguousarray(np.asarray(x, dtype=np.float32)
                             .reshape(B_TOT, C, HW))
    in_maps = []
    for core in range(N_CORES):
        in_maps.append({
            "x": x[core * B_LOC:(core + 1) * B_LOC],
            "wt": consts["wt"],
            "pack": consts["pack"],
        })
    res = run_bass_kernel_spmd(nc, in_maps, list(range(N_CORES)))
    out = np.concatenate([res.results[i]["out"] for i in range(N_CORES)],
                         axis=0)
    return out.reshape(B_TOT, C, H, W)


if __name__ == "__main__":
    rng = np.random.default_rng(0)
    x = rng.standard_normal((B_TOT, C, H, W), dtype=np.float32)
    out = kernel(x, np.ones(C, np.float32), np.zeros(C, np.float32),
                 np.float32(6.0),
                 rng.standard_normal((C, C, 3, 3), dtype=np.float32) * 0.03,
                 np.zeros(C, np.float32))
    print(out.shape, out.dtype)


# revision 36
# speedup vs baseline: 364.8999x; 1.0080x over previous
"""Trainium2 Bass kernel for BitConv2d:
GroupNorm(8) -> ReLU^2 -> PACT 8-bit quant -> 3x3 conv (ternary weight) -> bias.

Strategy (data-parallel over batch, 8 cores x 4 images):
 - Host: ternarize the [256,256,3,3] weight (exact forward value is
   alpha_oc * sign(w) * mask); keep only the {-1,0,+1} pattern for the device
   (fp16), folding alpha_oc/S into a per-out-channel rescale applied at PSUM
   evacuation time.
 - Device per image: GroupNorm stats via bn_stats (per-partition mean/var)
   + two tiny PE matmuls (group-reduce over 32-channel groups and broadcast
   back, with gamma folded into the broadcast matrix).  Then one ACT pass
   relu(A*x+B), one ACT pass Square(sqrt(S)*u), and one DVE pass
   min(.,255)+1024 whose fp32->fp16 output convert rounds to the integer
   quantization grid (RNE, even offset preserves jnp.round tie parity;
   HW-verified) writing into a 1024-padded fp16 [128,58,58] image.  Because
   the padding is also 1024, conv(n+1024) differs from conv(n) by a
   per-out-channel constant that is folded into the output bias.
 - Conv: out[co, 8x56 spatial block] accumulates 18 matmuls
   (9 taps x 2 cin chunks) of [128k x 128m] @ [128k x 448n] into PSUM;
   evacuation = ACT Identity(psum * scale_co + bias'_co) then DMA out.
All quantities n+1024 in [1024,1279] are exactly representable in fp16 and
the fp32 PSUM accumulation of <= 2304 such integer terms is exact, so the
conv is bit-exact integer arithmetic.
"""

import os
import sys

import numpy as np

for _p in ("/opt/trn_rl_repo", "/root/.axon_site/_ro/trn_rl_repo"):
    if _p not in sys.path and os.path.isdir(_p):
        sys.path.append(_p)

GN_GROUPS = 8
GN_EPS = 1e-5
K_BITS = 8
DELTA = 0.05
EPS = 1e-8

B_TOT, C, H, W = 32, 256, 56, 56
HW = H * W  # 3136
PW = W + 2  # 58
N_CORES = 8
B_LOC = B_TOT // N_CORES  # 4
RB = 7          # row blocks per image
RBH = H // RB   # 8 rows per block
NN = RBH * W    # 448 columns per conv matmul
C32 = 12582912.0  # 1.5 * 2**23, fp32 round-to-nearest-int trick
OFF = 1024.0      # fp16 integer-exact offset
ROUND_VIA_F16 = True  # round via fp32->fp16 RNE convert (1 DVE pass) instead
#                       of the +C32 fp32 trick (2 passes); HW-verified below


def _host_prep(gamma, beta, a, weight_fp, bias):
    """Ternarize weights and pack all small device constants (numpy)."""
    w = weight_fp.astype(np.float32)
    wv = w.reshape(C, -1)
    wa = np.abs(wv)
    t = (DELTA * wa.mean(axis=1, keepdims=True)).astype(np.float32)
    m = (wa > t)
    alpha = ((wa * m).sum(axis=1, dtype=np.float64)
             / (m.sum(axis=1).astype(np.float64) + EPS)).astype(np.float32)
    tern = (np.sign(wv) * m).astype(np.float32)  # [-1, 0, 1]
    t_co = tern.sum(axis=1)  # integer-valued, per out channel

    a_c = np.float32(max(float(a), 0.0) + EPS)
    S = np.float32((2.0 ** K_BITS - 1.0) / a_c)
    sqrt_s = float(np.float32(np.sqrt(np.float64(S))))

    # out = (alpha_co / S) * conv(n, tern) + bias_co
    # psum = conv(n + OFF, tern) = conv(n, tern) + OFF * t_co
    scale = (alpha / S).astype(np.float32)
    bias2 = (bias.astype(np.float64) - np.float64(OFF) * scale.astype(np.float64)
             * t_co.astype(np.float64)).astype(np.float32)

    # lhsT layout: wt[p, c, k, q, m] = tern[oc=q*128+m, ci=c*128+p, k]
    tern4 = tern.reshape(2, 128, 2, 128, 9)          # [q, m, c, p, k]
    wt = np.ascontiguousarray(tern4.transpose(3, 2, 4, 0, 1)).astype(np.float16)

    # one packed [128, 268] f32 constant tensor -> a single DMA:
    #   cols 0:4     ind (group-average matrix, 1/32 one-hot)
    #   cols 4:132   indtg chunk0 (rows 0..3 used: gamma-weighted one-hot.T)
    #   cols 132:260 indtg chunk1 (rows 0..3 used)
    #   cols 260:262 sc chunk0 (scale, bias')   cols 262:264 sc chunk1
    #   cols 264:266 gb chunk0 (gamma, beta)    cols 266:268 gb chunk1
    g32 = gamma.astype(np.float32)
    pack = np.zeros((128, 268), np.float32)
    pack[np.arange(128), np.arange(128) // 32] = 1.0 / 32.0
    for cch in range(2):
        for p in range(128):
            pack[p // 32, 4 + 128 * cch + p] = g32[cch * 128 + p]
        pack[:, 260 + 2 * cch] = scale.reshape(2, 128)[cch]
        pack[:, 261 + 2 * cch] = bias2.reshape(2, 128)[cch]
        pack[:, 264 + 2 * cch] = g32.reshape(2, 128)[cch]
        pack[:, 265 + 2 * cch] = beta.astype(np.float32).reshape(2, 128)[cch]

    return dict(wt=wt, pack=pack, sqrt_s=sqrt_s)


def _build_nc(sqrt_s, ablate=None):
    import concourse.bacc as bacc
    import concourse.mybir as mybir
    import concourse.tile as tile
    from contextlib import ExitStack

    f32 = mybir.dt.float32
    f16 = mybir.dt.float16
    AF = mybir.ActivationFunctionType
    ALU = mybir.AluOpType

    # Bacc (not raw Bass): its compile() runs generate_event_semaphores,
    # which legalizes multi-sem waits down to the 1-wait-per-instruction
    # TRN2 ISA constraint.
    nc = bacc.Bacc("TRN2", target_bir_lowering=False, debug=False,
                   num_devices=N_CORES)

    x_ap = nc.dram_tensor("x", [B_LOC, C, HW], f32, kind="ExternalInput").ap()
    wt_ap = nc.dram_tensor("wt", [128, 2, 9, 2, 128], f16,
                           kind="ExternalInput").ap()
    pack_ap = nc.dram_tensor("pack", [128, 268], f32,
                             kind="ExternalInput").ap()
    out_ap = nc.dram_tensor("out", [B_LOC, C, HW], f32,
                            kind="ExternalOutput").ap()

    with tile.TileContext(nc) as tc, ExitStack() as ctx:
        consts = ctx.enter_context(tc.tile_pool(name="consts", bufs=1))
        x_pool = ctx.enter_context(tc.tile_pool(name="xp", bufs=2))
        u_pool = ctx.enter_context(tc.tile_pool(name="up", bufs=2))
        pad_pool = ctx.enter_context(tc.tile_pool(name="padp", bufs=1))
        st_pool = ctx.enter_context(tc.tile_pool(name="stp", bufs=2))
        tiny = ctx.enter_context(tc.tile_pool(name="tinyp", bufs=4))
        out_pool = ctx.enter_context(tc.tile_pool(name="outp", bufs=6))
        cps_pool = ctx.enter_context(tc.tile_pool(name="cps", bufs=4,
                                                  space="PSUM"))
        gps_pool = ctx.enter_context(tc.tile_pool(name="gps", bufs=2,
                                                  space="PSUM"))

        xs = [[None] * 2 for _ in range(B_LOC)]
        gms = [[None] * 2 for _ in range(B_LOC)]
        abs_ = [[None] * 2 for _ in range(B_LOC)]

        def emit_load_chunk(b, cch, bounds=(0, HW)):
            # split loads (at bn_stats 448-block boundaries) so stats on
            # early blocks start before the tail lands (Tile subtile deps)
            xt = x_pool.tile([128, HW], f32, name=f"x_{b}_{cch}",
                             tag=f"x{cch}")
            for lo, hi in zip(bounds[:-1], bounds[1:]):
                nc.sync.dma_start(
                    out=xt[:, lo:hi],
                    in_=x_ap[b, cch * 128:(cch + 1) * 128, lo:hi])
            xs[b][cch] = xt

        def emit_load(b, bounds=(0, HW)):
            for cch in range(2):
                emit_load_chunk(b, cch, bounds)

        # image 0 is on the critical path: chunk 0's x pieces dispatch first
        # (HWDGE dispatch is ~625ns per dma_start and serializes), then the
        # packed small consts, then chunk 1, then the conv weights
        emit_load_chunk(0, 0, bounds=(0, 3 * NN, 6 * NN, HW))
        pk_sb = consts.tile([128, 268], f32, name="pk_sb")
        nc.sync.dma_start(out=pk_sb, in_=pack_ap)
        emit_load_chunk(0, 1, bounds=(0, 3 * NN, 6 * NN, HW))

        w_sb = consts.tile([128, 2, 9, 2, 128], f16, name="w_sb")
        nc.sync.dma_start(out=w_sb, in_=wt_ap)

        ind_sb = pk_sb[:, 0:4]
        indtg_sb = [pk_sb[0:4, 4:132], pk_sb[0:4, 132:260]]
        sc_sb = [pk_sb[:, 260:262], pk_sb[:, 262:264]]
        gb_sb = [pk_sb[:, 264:266], pk_sb[:, 266:268]]
        eps_sb = consts.tile([4, 1], f32, name="eps_sb")
        nc.vector.memset(eps_sb, GN_EPS)

        pad_t = [[pad_pool.tile([128, PW, PW], f16, name=f"pad_{cch}_{s}",
                                tag=f"pad{cch}{s}")
                  for s in range(2)] for cch in range(2)]
        for cch in range(2):
            for s in range(2):
                # border-only memsets (interior is overwritten every image),
                # on GPSIMD which is otherwise idle — a full-tile DVE memset
                # costs 3.5us x4 and sits right in front of bn_stats
                pt = pad_t[cch][s]
                nc.gpsimd.memset(pt[:, 0, :], OFF)
                nc.gpsimd.memset(pt[:, PW - 1, :], OFF)
                nc.gpsimd.memset(pt[:, 1:PW - 1, 0:1], OFF)
                nc.gpsimd.memset(pt[:, 1:PW - 1, PW - 1:PW], OFF)

        def emit_stats(b, chunks=(0, 1)):
            for cch in chunks:
                st6 = st_pool.tile([128, RB, 6], f32, name=f"st6_{b}_{cch}",
                                   tag="st6")
                for j in range(RB):
                    nc.vector.bn_stats(out=st6[:, j, :],
                                       in_=xs[b][cch][:, j * NN:(j + 1) * NN])
                mv = tiny.tile([128, 2], f32, name=f"mv_{b}_{cch}", tag="mv")
                nc.vector.bn_aggr(out=mv, in_=st6)
                msq = tiny.tile([128, 1], f32, name=f"msq_{b}_{cch}", tag="msq")
                nc.vector.tensor_mul(msq, mv[:, 0:1], mv[:, 0:1])
                m2 = tiny.tile([128, 2], f32, name=f"m2_{b}_{cch}", tag="m2")
                # keep every matmul-operand producer on DVE: Matmult's
                # LDWEIGHTS slot only fits 2 sync waits (walrus NCC_INLA001)
                nc.vector.tensor_copy(out=m2[:, 0:1], in_=mv[:, 0:1])
                nc.vector.tensor_add(m2[:, 1:2], mv[:, 1:2], msq)
                gmv = gps_pool.tile([4, 2], f32, name=f"gmv_{b}_{cch}",
                                    tag="gmv")
                nc.tensor.matmul(gmv, lhsT=ind_sb, rhs=m2, start=True,
                                 stop=True)
                gm = tiny.tile([4, 2], f32, name=f"gm_{b}_{cch}", tag="gm")
                nc.vector.tensor_copy(out=gm, in_=gmv)
                gms[b][cch] = gm

        def emit_post(b, chunks=(0, 1)):
            for cch in chunks:
                gm = gms[b][cch]
                gsq = tiny.tile([4, 1], f32, name=f"gsq_{b}_{cch}", tag="gsq")
                nc.vector.tensor_mul(gsq, gm[:, 0:1], gm[:, 0:1])
                gvar = tiny.tile([4, 1], f32, name=f"gvar_{b}_{cch}",
                                 tag="gvar")
                nc.vector.tensor_sub(gvar, gm[:, 1:2], gsq)
                # gvar <- 1/sqrt(var + eps)
                nc.scalar.activation(out=gvar, in_=gvar, func=AF.Sqrt,
                                     bias=eps_sb, scale=1.0)
                nc.vector.reciprocal(out=gvar, in_=gvar)
                vals = tiny.tile([4, 2], f32, name=f"vals_{b}_{cch}",
                                 tag="vals")
                nc.vector.tensor_mul(vals[:, 0:1], gm[:, 0:1], gvar)
                nc.vector.tensor_copy(out=vals[:, 1:2], in_=gvar)
                bc = gps_pool.tile([128, 2], f32, name=f"bc_{b}_{cch}",
                                   tag="bc")
                nc.tensor.matmul(bc, lhsT=indtg_sb[cch], rhs=vals, start=True,
                                 stop=True)
                ab = tiny.tile([128, 2], f32, name=f"ab_{b}_{cch}", tag="ab")
                nc.vector.tensor_copy(out=ab, in_=bc)
                bt = tiny.tile([128, 1], f32, name=f"bt_{b}_{cch}", tag="bt")
                nc.vector.tensor_sub(bt, gb_sb[cch][:, 1:2], ab[:, 0:1])
                abs_[b][cch] = (ab, bt)

        def emit_act(b, split=1, chunks=(0, 1), dve_sq=False):
            # split>1 shortens chain latency (image 0 is on the critical
            # path before any conv work exists); dve_sq moves alternate
            # quarters' square onto DVE to unbottleneck ACT at startup
            nh = split
            hrows = H // nh
            hlen = hrows * W
            for cch in chunks:
                ab, bt = abs_[b][cch]
                u = u_pool.tile([128, HW], f32, name=f"u_{b}_{cch}",
                                tag=f"u{cch}")
                for h in range(nh):
                    sl = slice(h * hlen, (h + 1) * hlen)
                    uv = u[:, sl]
                    # u = relu(A*x + B)
                    nc.scalar.activation(out=uv, in_=xs[b][cch][:, sl],
                                         func=AF.Relu,
                                         bias=bt, scale=ab[:, 1:2])
                    if dve_sq and (h + cch) % 2 == 1:
                        # u = (u*u)*S on DVE (tensor_tensor_reduce)
                        tacc = tiny.tile([128, 1], f32,
                                         name=f"tacc_{b}_{cch}_{h}",
                                         tag="tacc")
                        nc.vector.tensor_tensor_reduce(
                            out=uv, in0=uv, in1=uv, scale=sqrt_s * sqrt_s,
                            scalar=0.0, op0=ALU.mult, op1=ALU.max,
                            accum_out=tacc)
                    else:
                        # u = (sqrt(S)*u)^2 = S*relu(z)^2
                        nc.scalar.activation(out=uv, in_=uv, func=AF.Square,
                                             scale=sqrt_s)
                    if ROUND_VIA_F16:
                        # pad interior = min(u,255) + 1024; the fp32->fp16
                        # output convert rounds to the integer grid (RNE;
                        # +1024 is even so tie parity matches jnp.round)
                        nc.vector.tensor_scalar(
                            out=pad_t[cch][b % 2][:, 1 + h * hrows:
                                                  1 + (h + 1) * hrows,
                                                  1:W + 1],
                            in0=uv.rearrange("p (h w) -> p h w", h=hrows),
                            scalar1=255.0, scalar2=OFF,
                            op0=ALU.min, op1=ALU.add)
                    else:
                        # u = min(u, 255) + C32 (fp32 add rounds to int, RNE)
                        nc.vector.tensor_scalar(out=uv, in0=uv, scalar1=255.0,
                                                scalar2=C32, op0=ALU.min,
                                                op1=ALU.add)
                        # pad interior = (u - C32) + 1024, exact f16 integers
                        nc.vector.tensor_scalar_add(
                            out=pad_t[cch][b % 2][:, 1 + h * hrows:
                                                  1 + (h + 1) * hrows,
                                                  1:W + 1],
                            in0=uv.rearrange("p (h w) -> p h w", h=hrows),
                            scalar1=OFF - C32)

        def emit_conv(b, q):
            # one full-image output tile per (b, q): evacs write slices and a
            # single big DMA stores it (HWDGE dispatch is ~625ns per
            # dma_start; 7 small stores would serialize against the loads)
            osb = out_pool.tile([128, HW], f32, name=f"o_{b}_{q}", tag="osb",
                                bufs=2)
            for rb in range(RB):
                ps = cps_pool.tile([128, NN], f32, name=f"ps_{b}_{q}_{rb}",
                                   tag="cps")
                i = 0
                for cch in range(2):  # cch-outer: taps of chunk 0 can start
                    for k in range(9):  # before chunk 1's pad is written
                        dy, dx = divmod(k, 3)
                        nc.tensor.matmul(
                            ps,
                            lhsT=w_sb[:, cch, k, q, :],
                            rhs=pad_t[cch][b % 2][:, rb * RBH + dy:
                                                  rb * RBH + dy + RBH,
                                                  dx:dx + W],
                            start=(i == 0), stop=(i == 17))
                        i += 1
                nc.scalar.activation(out=osb[:, rb * NN:(rb + 1) * NN],
                                     in_=ps, func=AF.Identity,
                                     bias=sc_sb[q][:, 1:2],
                                     scale=sc_sb[q][:, 0:1])
            # piecewise stores: each piece only depends on the evacs that
            # wrote it (subtile deps).  The last image streams out
            # per-rowblock so the kernel tail isn't gated on one big DMA.
            cuts = (list(range(0, HW + 1, NN)) if b == B_LOC - 1
                    else [0, 4 * NN, HW])
            for lo, hi in zip(cuts[:-1], cuts[1:]):
                nc.sync.dma_start(out=out_ap[b, q * 128:(q + 1) * 128, lo:hi],
                                  in_=osb[:, lo:hi])

        if ablate == "conv":
            for b in range(B_LOC):
                emit_conv(b, 0)
                emit_conv(b, 1)
        elif ablate == "gn":
            emit_stats(0); emit_post(0); emit_act(0)
            for b in range(1, B_LOC):
                emit_load(b); emit_stats(b); emit_post(b); emit_act(b)
        else:
            for cch in range(2):
                emit_stats(0, chunks=(cch,))
                emit_post(0, chunks=(cch,))
                emit_act(0, split=4, chunks=(cch,))
            for b in range(B_LOC):
                if b + 1 < B_LOC:
                    emit_load(b + 1)
                    emit_stats(b + 1)
                emit_conv(b, 0)
                if b + 1 < B_LOC:
                    emit_post(b + 1)
                emit_conv(b, 1)
                if b + 1 < B_LOC:
                    emit_act(b + 1)

    nc.compile()
    return nc


def kernel(x, gamma, beta, a, weight_fp, bias):
    consts = _host_prep(np.asarray(gamma), np.asarray(beta), np.asarray(a),
                        np.asarray(weight_fp), np.asarray(bias))
    nc = _build_nc(consts.pop("sqrt_s"))

    from concourse.bass_utils import run_bass_kernel_spmd

    x = np.ascontiguousarray(np.asarray(x, dtype=np.float32)
                             .reshape(B_TOT, C, HW))
    in_maps = []
    for core in range(N_CORES):
        in_maps.append({
            "x": x[core * B_LOC:(core + 1) * B_LOC],
            "wt": consts["wt"],
            "pack": consts["pack"],
        })
    res = run_bass_kernel_spmd(nc, in_maps, list(range(N_CORES)))
    out = np.concatenate([res.results[i]["out"] for i in range(N_CORES)],
                         axis=0)
    return out.reshape(B_TOT, C, H, W)


if __name__ == "__main__":
    rng = np.random.default_rng(0)
    x = rng.standard_normal((B_TOT, C, H, W), dtype=np.float32)
    out = kernel(x, np.ones(C, np.float32), np.zeros(C, np.float32),
                 np.float32(6.0),
                 rng.standard_normal((C, C, 3, 3), dtype=np.float32) * 0.03,
                 np.zeros(C, np.float32))
    print(out.shape, out.dtype)


# revision 38
# speedup vs baseline: 365.3870x; 1.0013x over previous
"""Trainium2 Bass kernel for BitConv2d:
GroupNorm(8) -> ReLU^2 -> PACT 8-bit quant -> 3x3 conv (ternary weight) -> bias.

Strategy (data-parallel over batch, 8 cores x 4 images):
 - Host: ternarize the [256,256,3,3] weight (exact forward value is
   alpha_oc * sign(w) * mask); keep only the {-1,0,+1} pattern for the device
   (fp16), folding alpha_oc/S into a per-out-channel rescale applied at PSUM
   evacuation time.
 - Device per image: GroupNorm stats via bn_stats (per-partition mean/var)
   + two tiny PE matmuls (group-reduce over 32-channel groups and broadcast
   back, with gamma folded into the broadcast matrix).  Then one ACT pass
   relu(A*x+B), one ACT pass Square(sqrt(S)*u), and one DVE pass
   min(.,255)+1024 whose fp32->fp16 output convert rounds to the integer
   quantization grid (RNE, even offset preserves jnp.round tie parity;
   HW-verified) writing into a 1024-padded fp16 [128,58,58] image.  Because
   the padding is also 1024, conv(n+1024) differs from conv(n) by a
   per-out-channel constant that is folded into the output bias.
 - Conv: out[co, 8x56 spatial block] accumulates 18 matmuls
   (9 taps x 2 cin chunks) of [128k x 128m] @ [128k x 448n] into PSUM;
   evacuation = ACT Identity(psum * scale_co + bias'_co) then DMA out.
All quantities n+1024 in [1024,1279] are exactly representable in fp16 and
the fp32 PSUM accumulation of <= 2304 such integer terms is exact, so the
conv is bit-exact integer arithmetic.
"""

import os
import sys

import numpy as np

for _p in ("/opt/trn_rl_repo", "/root/.axon_site/_ro/trn_rl_repo"):
    if _p not in sys.path and os.path.isdir(_p):
        sys.path.append(_p)

GN_GROUPS = 8
GN_EPS = 1e-5
K_BITS = 8
DELTA = 0.05
EPS = 1e-8

B_TOT, C, H, W = 32, 256, 56, 56
HW = H * W  # 3136
PW = W + 2  # 58
N_CORES = 8
B_LOC = B_TOT // N_CORES  # 4
RB = 7          # row blocks per image
RBH = H // RB   # 8 rows per block
NN = RBH * W    # 448 columns per conv matmul
C32 = 12582912.0  # 1.5 * 2**23, fp32 round-to-nearest-int trick
OFF = 1024.0      # fp16 integer-exact offset
ROUND_VIA_F16 = True  # round via fp32->fp16 RNE convert (1 DVE pass) instead
#                       of the +C32 fp32 trick (2 passes); HW-verified below


def _host_prep(gamma, beta, a, weight_fp, bias):
    """Ternarize weights and pack all small device constants (numpy)."""
    w = weight_fp.astype(np.float32)
    wv = w.reshape(C, -1)
    wa = np.abs(wv)
    t = (DELTA * wa.mean(axis=1, keepdims=True)).astype(np.float32)
    m = (wa > t)
    alpha = ((wa * m).sum(axis=1, dtype=np.float64)
             / (m.sum(axis=1).astype(np.float64) + EPS)).astype(np.float32)
    tern = (np.sign(wv) * m).astype(np.float32)  # [-1, 0, 1]
    t_co = tern.sum(axis=1)  # integer-valued, per out channel

    a_c = np.float32(max(float(a), 0.0) + EPS)
    S = np.float32((2.0 ** K_BITS - 1.0) / a_c)
    sqrt_s = float(np.float32(np.sqrt(np.float64(S))))

    # out = (alpha_co / S) * conv(n, tern) + bias_co
    # psum = conv(n + OFF, tern) = conv(n, tern) + OFF * t_co
    scale = (alpha / S).astype(np.float32)
    bias2 = (bias.astype(np.float64) - np.float64(OFF) * scale.astype(np.float64)
             * t_co.astype(np.float64)).astype(np.float32)

    # lhsT layout: wt[p, c, k, q, m] = tern[oc=q*128+m, ci=c*128+p, k]
    tern4 = tern.reshape(2, 128, 2, 128, 9)          # [q, m, c, p, k]
    wt = np.ascontiguousarray(tern4.transpose(3, 2, 4, 0, 1)).astype(np.float16)

    # one packed [128, 268] f32 constant tensor -> a single DMA:
    #   cols 0:4     ind (group-average matrix, 1/32 one-hot)
    #   cols 4:132   indtg chunk0 (rows 0..3 used: gamma-weighted one-hot.T)
    #   cols 132:260 indtg chunk1 (rows 0..3 used)
    #   cols 260:262 sc chunk0 (scale, bias')   cols 262:264 sc chunk1
    #   cols 264:266 gb chunk0 (gamma, beta)    cols 266:268 gb chunk1
    g32 = gamma.astype(np.float32)
    pack = np.zeros((128, 268), np.float32)
    pack[np.arange(128), np.arange(128) // 32] = 1.0 / 32.0
    for cch in range(2):
        for p in range(128):
            pack[p // 32, 4 + 128 * cch + p] = g32[cch * 128 + p]
        pack[:, 260 + 2 * cch] = scale.reshape(2, 128)[cch]
        pack[:, 261 + 2 * cch] = bias2.reshape(2, 128)[cch]
        pack[:, 264 + 2 * cch] = g32.reshape(2, 128)[cch]
        pack[:, 265 + 2 * cch] = beta.astype(np.float32).reshape(2, 128)[cch]

    return dict(wt=wt, pack=pack, sqrt_s=sqrt_s)


def _build_nc(sqrt_s, ablate=None):
    import concourse.bacc as bacc
    import concourse.mybir as mybir
    import concourse.tile as tile
    from contextlib import ExitStack

    f32 = mybir.dt.float32
    f16 = mybir.dt.float16
    AF = mybir.ActivationFunctionType
    ALU = mybir.AluOpType

    # Bacc (not raw Bass): its compile() runs generate_event_semaphores,
    # which legalizes multi-sem waits down to the 1-wait-per-instruction
    # TRN2 ISA constraint.
    nc = bacc.Bacc("TRN2", target_bir_lowering=False, debug=False,
                   num_devices=N_CORES)

    x_ap = nc.dram_tensor("x", [B_LOC, C, HW], f32, kind="ExternalInput").ap()
    wt_ap = nc.dram_tensor("wt", [128, 2, 9, 2, 128], f16,
                           kind="ExternalInput").ap()
    pack_ap = nc.dram_tensor("pack", [128, 268], f32,
                             kind="ExternalInput").ap()
    out_ap = nc.dram_tensor("out", [B_LOC, C, HW], f32,
                            kind="ExternalOutput").ap()

    with tile.TileContext(nc) as tc, ExitStack() as ctx:
        consts = ctx.enter_context(tc.tile_pool(name="consts", bufs=1))
        x_pool = ctx.enter_context(tc.tile_pool(name="xp", bufs=2))
        u_pool = ctx.enter_context(tc.tile_pool(name="up", bufs=2))
        pad_pool = ctx.enter_context(tc.tile_pool(name="padp", bufs=1))
        st_pool = ctx.enter_context(tc.tile_pool(name="stp", bufs=2))
        tiny = ctx.enter_context(tc.tile_pool(name="tinyp", bufs=4))
        out_pool = ctx.enter_context(tc.tile_pool(name="outp", bufs=6))
        cps_pool = ctx.enter_context(tc.tile_pool(name="cps", bufs=4,
                                                  space="PSUM"))
        gps_pool = ctx.enter_context(tc.tile_pool(name="gps", bufs=2,
                                                  space="PSUM"))

        xs = [[None] * 2 for _ in range(B_LOC)]
        gms = [[None] * 2 for _ in range(B_LOC)]
        abs_ = [[None] * 2 for _ in range(B_LOC)]

        def emit_load_chunk(b, cch, bounds=(0, HW)):
            # split loads (at bn_stats 448-block boundaries) so stats on
            # early blocks start before the tail lands (Tile subtile deps)
            xt = x_pool.tile([128, HW], f32, name=f"x_{b}_{cch}",
                             tag=f"x{cch}")
            for lo, hi in zip(bounds[:-1], bounds[1:]):
                nc.sync.dma_start(
                    out=xt[:, lo:hi],
                    in_=x_ap[b, cch * 128:(cch + 1) * 128, lo:hi])
            xs[b][cch] = xt

        def emit_load(b, bounds=(0, HW)):
            for cch in range(2):
                emit_load_chunk(b, cch, bounds)

        # image 0 is on the critical path: chunk 0's x pieces dispatch first
        # (HWDGE dispatch is ~625ns per dma_start and serializes), then the
        # packed small consts, then chunk 1, then the conv weights
        emit_load_chunk(0, 0, bounds=(0, 3 * NN, 6 * NN, HW))
        pk_sb = consts.tile([128, 268], f32, name="pk_sb")
        nc.sync.dma_start(out=pk_sb, in_=pack_ap)
        emit_load_chunk(0, 1, bounds=(0, 3 * NN, 6 * NN, HW))

        w_sb = consts.tile([128, 2, 9, 2, 128], f16, name="w_sb")
        nc.sync.dma_start(out=w_sb, in_=wt_ap)

        ind_sb = pk_sb[:, 0:4]
        indtg_sb = [pk_sb[0:4, 4:132], pk_sb[0:4, 132:260]]
        sc_sb = [pk_sb[:, 260:262], pk_sb[:, 262:264]]
        gb_sb = [pk_sb[:, 264:266], pk_sb[:, 266:268]]
        eps_sb = consts.tile([4, 1], f32, name="eps_sb")
        nc.vector.memset(eps_sb, GN_EPS)

        pad_t = [[pad_pool.tile([128, PW, PW], f16, name=f"pad_{cch}_{s}",
                                tag=f"pad{cch}{s}")
                  for s in range(2)] for cch in range(2)]
        for cch in range(2):
            for s in range(2):
                # border-only memsets (interior is overwritten every image),
                # on GPSIMD which is otherwise idle — a full-tile DVE memset
                # costs 3.5us x4 and sits right in front of bn_stats
                pt = pad_t[cch][s]
                nc.gpsimd.memset(pt[:, 0, :], OFF)
                nc.gpsimd.memset(pt[:, PW - 1, :], OFF)
                nc.gpsimd.memset(pt[:, 1:PW - 1, 0:1], OFF)
                nc.gpsimd.memset(pt[:, 1:PW - 1, PW - 1:PW], OFF)

        def emit_stats(b, chunks=(0, 1)):
            for cch in chunks:
                st6 = st_pool.tile([128, RB, 6], f32, name=f"st6_{b}_{cch}",
                                   tag="st6")
                for j in range(RB):
                    nc.vector.bn_stats(out=st6[:, j, :],
                                       in_=xs[b][cch][:, j * NN:(j + 1) * NN])
                # rhs = (mean, var, mean^2) built by slice-writes (keeps
                # every matmul-operand producer on DVE: Matmult's LDWEIGHTS
                # slot only fits 2 sync waits, walrus NCC_INLA001)
                m3 = tiny.tile([128, 3], f32, name=f"m3_{b}_{cch}", tag="m3")
                nc.vector.bn_aggr(out=m3[:, 0:2], in_=st6)
                nc.vector.tensor_mul(m3[:, 2:3], m3[:, 0:1], m3[:, 0:1])
                gmv = gps_pool.tile([4, 3], f32, name=f"gmv_{b}_{cch}",
                                    tag="gmv")
                nc.tensor.matmul(gmv, lhsT=ind_sb, rhs=m3, start=True,
                                 stop=True)
                gm = tiny.tile([4, 3], f32, name=f"gm_{b}_{cch}", tag="gm")
                nc.vector.tensor_copy(out=gm, in_=gmv)
                gms[b][cch] = gm

        def emit_post(b, chunks=(0, 1)):
            for cch in chunks:
                gm = gms[b][cch]
                gsq = tiny.tile([4, 1], f32, name=f"gsq_{b}_{cch}", tag="gsq")
                nc.vector.tensor_mul(gsq, gm[:, 0:1], gm[:, 0:1])
                gvar = tiny.tile([4, 1], f32, name=f"gvar_{b}_{cch}",
                                 tag="gvar")
                # var_g = (avg var + avg mean^2) - mean_g^2, one fused op
                nc.vector.scalar_tensor_tensor(
                    out=gvar, in0=gm[:, 1:2], scalar=gm[:, 2:3], in1=gsq,
                    op0=ALU.add, op1=ALU.subtract)
                # gvar <- 1/sqrt(var + eps); reciprocal lands straight in
                # vals[:,1] to skip a copy on the critical chain
                nc.scalar.activation(out=gvar, in_=gvar, func=AF.Sqrt,
                                     bias=eps_sb, scale=1.0)
                vals = tiny.tile([4, 2], f32, name=f"vals_{b}_{cch}",
                                 tag="vals")
                nc.vector.reciprocal(out=vals[:, 1:2], in_=gvar)
                nc.vector.tensor_mul(vals[:, 0:1], gm[:, 0:1], vals[:, 1:2])
                bc = gps_pool.tile([128, 2], f32, name=f"bc_{b}_{cch}",
                                   tag="bc")
                nc.tensor.matmul(bc, lhsT=indtg_sb[cch], rhs=vals, start=True,
                                 stop=True)
                ab = tiny.tile([128, 2], f32, name=f"ab_{b}_{cch}", tag="ab")
                nc.vector.tensor_copy(out=ab, in_=bc)
                bt = tiny.tile([128, 1], f32, name=f"bt_{b}_{cch}", tag="bt")
                nc.vector.tensor_sub(bt, gb_sb[cch][:, 1:2], ab[:, 0:1])
                abs_[b][cch] = (ab, bt)

        def emit_act(b, split=1, chunks=(0, 1), dve_sq=False):
            # split>1 shortens chain latency (image 0 is on the critical
            # path before any conv work exists); dve_sq moves alternate
            # quarters' square onto DVE to unbottleneck ACT at startup
            nh = split
            hrows = H // nh
            hlen = hrows * W
            for cch in chunks:
                ab, bt = abs_[b][cch]
                u = u_pool.tile([128, HW], f32, name=f"u_{b}_{cch}",
                                tag=f"u{cch}")
                for h in range(nh):
                    sl = slice(h * hlen, (h + 1) * hlen)
                    uv = u[:, sl]
                    # u = relu(A*x + B)
                    nc.scalar.activation(out=uv, in_=xs[b][cch][:, sl],
                                         func=AF.Relu,
                                         bias=bt, scale=ab[:, 1:2])
                    if dve_sq and (h + cch) % 2 == 1:
                        # u = (u*u)*S on DVE (tensor_tensor_reduce)
                        tacc = tiny.tile([128, 1], f32,
                                         name=f"tacc_{b}_{cch}_{h}",
                                         tag="tacc")
                        nc.vector.tensor_tensor_reduce(
                            out=uv, in0=uv, in1=uv, scale=sqrt_s * sqrt_s,
                            scalar=0.0, op0=ALU.mult, op1=ALU.max,
                            accum_out=tacc)
                    else:
                        # u = (sqrt(S)*u)^2 = S*relu(z)^2
                        nc.scalar.activation(out=uv, in_=uv, func=AF.Square,
                                             scale=sqrt_s)
                    if ROUND_VIA_F16:
                        # pad interior = min(u,255) + 1024; the fp32->fp16
                        # output convert rounds to the integer grid (RNE;
                        # +1024 is even so tie parity matches jnp.round)
                        nc.vector.tensor_scalar(
                            out=pad_t[cch][b % 2][:, 1 + h * hrows:
                                                  1 + (h + 1) * hrows,
                                                  1:W + 1],
                            in0=uv.rearrange("p (h w) -> p h w", h=hrows),
                            scalar1=255.0, scalar2=OFF,
                            op0=ALU.min, op1=ALU.add)
                    else:
                        # u = min(u, 255) + C32 (fp32 add rounds to int, RNE)
                        nc.vector.tensor_scalar(out=uv, in0=uv, scalar1=255.0,
                                                scalar2=C32, op0=ALU.min,
                                                op1=ALU.add)
                        # pad interior = (u - C32) + 1024, exact f16 integers
                        nc.vector.tensor_scalar_add(
                            out=pad_t[cch][b % 2][:, 1 + h * hrows:
                                                  1 + (h + 1) * hrows,
                                                  1:W + 1],
                            in0=uv.rearrange("p (h w) -> p h w", h=hrows),
                            scalar1=OFF - C32)

        def emit_conv(b, q):
            # one full-image output tile per (b, q): evacs write slices and a
            # single big DMA stores it (HWDGE dispatch is ~625ns per
            # dma_start; 7 small stores would serialize against the loads)
            osb = out_pool.tile([128, HW], f32, name=f"o_{b}_{q}", tag="osb",
                                bufs=2)
            for rb in range(RB):
                ps = cps_pool.tile([128, NN], f32, name=f"ps_{b}_{q}_{rb}",
                                   tag="cps")
                i = 0
                for cch in range(2):  # cch-outer: taps of chunk 0 can start
                    for k in range(9):  # before chunk 1's pad is written
                        dy, dx = divmod(k, 3)
                        nc.tensor.matmul(
                            ps,
                            lhsT=w_sb[:, cch, k, q, :],
                            rhs=pad_t[cch][b % 2][:, rb * RBH + dy:
                                                  rb * RBH + dy + RBH,
                                                  dx:dx + W],
                            start=(i == 0), stop=(i == 17))
                        i += 1
                nc.scalar.activation(out=osb[:, rb * NN:(rb + 1) * NN],
                                     in_=ps, func=AF.Identity,
                                     bias=sc_sb[q][:, 1:2],
                                     scale=sc_sb[q][:, 0:1])
            # piecewise stores: each piece only depends on the evacs that
            # wrote it (subtile deps).  The last image streams out
            # per-rowblock so the kernel tail isn't gated on one big DMA.
            cuts = (list(range(0, HW + 1, NN)) if b == B_LOC - 1
                    else [0, 4 * NN, HW])
            for lo, hi in zip(cuts[:-1], cuts[1:]):
                nc.sync.dma_start(out=out_ap[b, q * 128:(q + 1) * 128, lo:hi],
                                  in_=osb[:, lo:hi])

        if ablate == "conv":
            for b in range(B_LOC):
                emit_conv(b, 0)
                emit_conv(b, 1)
        elif ablate == "gn":
            emit_stats(0); emit_post(0); emit_act(0)
            for b in range(1, B_LOC):
                emit_load(b); emit_stats(b); emit_post(b); emit_act(b)
        else:
            for cch in range(2):
                emit_stats(0, chunks=(cch,))
                emit_post(0, chunks=(cch,))
                emit_act(0, split=4, chunks=(cch,))
            for b in range(B_LOC):
                if b + 1 < B_LOC:
                    emit_load(b + 1)
                    emit_stats(b + 1)
                emit_conv(b, 0)
                if b + 1 < B_LOC:
                    emit_post(b + 1)
                emit_conv(b, 1)
                if b + 1 < B_LOC:
                    emit_act(b + 1)

    nc.compile()
    return nc


def kernel(x, gamma, beta, a, weight_fp, bias):
    consts = _host_prep(np.asarray(gamma), np.asarray(beta), np.asarray(a),
                        np.asarray(weight_fp), np.asarray(bias))
    nc = _build_nc(consts.pop("sqrt_s"))

    from concourse.bass_utils import run_bass_kernel_spmd

    x = np.ascontiguousarray(np.asarray(x, dtype=np.float32)
                             .reshape(B_TOT, C, HW))
    in_maps = []
    for core in range(N_CORES):
        in_maps.append({
            "x": x[core * B_LOC:(core + 1) * B_LOC],
            "wt": consts["wt"],
            "pack": consts["pack"],
        })
    res = run_bass_kernel_spmd(nc, in_maps, list(range(N_CORES)))
    out = np.concatenate([res.results[i]["out"] for i in range(N_CORES)],
                         axis=0)
    return out.reshape(B_TOT, C, H, W)


if __name__ == "__main__":
    rng = np.random.default_rng(0)
    x = rng.standard_normal((B_TOT, C, H, W), dtype=np.float32)
    out = kernel(x, np.ones(C, np.float32), np.zeros(C, np.float32),
                 np.float32(6.0),
                 rng.standard_normal((C, C, 3, 3), dtype=np.float32) * 0.03,
                 np.zeros(C, np.float32))
    print(out.shape, out.dtype)


# revision 39
# speedup vs baseline: 366.3003x; 1.0025x over previous
"""Trainium2 Bass kernel for BitConv2d:
GroupNorm(8) -> ReLU^2 -> PACT 8-bit quant -> 3x3 conv (ternary weight) -> bias.

Strategy (data-parallel over batch, 8 cores x 4 images):
 - Host: ternarize the [256,256,3,3] weight (exact forward value is
   alpha_oc * sign(w) * mask); keep only the {-1,0,+1} pattern for the device
   (fp16), folding alpha_oc/S into a per-out-channel rescale applied at PSUM
   evacuation time.
 - Device per image: GroupNorm stats via bn_stats (per-partition mean/var)
   + two tiny PE matmuls (group-reduce over 32-channel groups and broadcast
   back, with gamma folded into the broadcast matrix).  Then one ACT pass
   relu(A*x+B), one ACT pass Square(sqrt(S)*u), and one DVE pass
   min(.,255)+1024 whose fp32->fp16 output convert rounds to the integer
   quantization grid (RNE, even offset preserves jnp.round tie parity;
   HW-verified) writing into a 1024-padded fp16 [128,58,58] image.  Because
   the padding is also 1024, conv(n+1024) differs from conv(n) by a
   per-out-channel constant that is folded into the output bias.
 - Conv: out[co, 8x56 spatial block] accumulates 18 matmuls
   (9 taps x 2 cin chunks) of [128k x 128m] @ [128k x 448n] into PSUM;
   evacuation = ACT Identity(psum * scale_co + bias'_co) then DMA out.
All quantities n+1024 in [1024,1279] are exactly representable in fp16 and
the fp32 PSUM accumulation of <= 2304 such integer terms is exact, so the
conv is bit-exact integer arithmetic.
"""

import os
import sys

import numpy as np

for _p in ("/opt/trn_rl_repo", "/root/.axon_site/_ro/trn_rl_repo"):
    if _p not in sys.path and os.path.isdir(_p):
        sys.path.append(_p)

GN_GROUPS = 8
GN_EPS = 1e-5
K_BITS = 8
DELTA = 0.05
EPS = 1e-8

B_TOT, C, H, W = 32, 256, 56, 56
HW = H * W  # 3136
PW = W + 2  # 58
N_CORES = 8
B_LOC = B_TOT // N_CORES  # 4
RB = 7          # row blocks per image
RBH = H // RB   # 8 rows per block
NN = RBH * W    # 448 columns per conv matmul
C32 = 12582912.0  # 1.5 * 2**23, fp32 round-to-nearest-int trick
OFF = 1024.0      # fp16 integer-exact offset
ROUND_VIA_F16 = True  # round via fp32->fp16 RNE convert (1 DVE pass) instead
#                       of the +C32 fp32 trick (2 passes); HW-verified below


def _host_prep(gamma, beta, a, weight_fp, bias):
    """Ternarize weights and pack all small device constants (numpy)."""
    w = weight_fp.astype(np.float32)
    wv = w.reshape(C, -1)
    wa = np.abs(wv)
    t = (DELTA * wa.mean(axis=1, keepdims=True)).astype(np.float32)
    m = (wa > t)
    alpha = ((wa * m).sum(axis=1, dtype=np.float64)
             / (m.sum(axis=1).astype(np.float64) + EPS)).astype(np.float32)
    tern = (np.sign(wv) * m).astype(np.float32)  # [-1, 0, 1]
    t_co = tern.sum(axis=1)  # integer-valued, per out channel

    a_c = np.float32(max(float(a), 0.0) + EPS)
    S = np.float32((2.0 ** K_BITS - 1.0) / a_c)
    sqrt_s = float(np.float32(np.sqrt(np.float64(S))))

    # out = (alpha_co / S) * conv(n, tern) + bias_co
    # psum = conv(n + OFF, tern) = conv(n, tern) + OFF * t_co
    scale = (alpha / S).astype(np.float32)
    bias2 = (bias.astype(np.float64) - np.float64(OFF) * scale.astype(np.float64)
             * t_co.astype(np.float64)).astype(np.float32)

    # lhsT layout: wt[p, c, k, q, m] = tern[oc=q*128+m, ci=c*128+p, k]
    tern4 = tern.reshape(2, 128, 2, 128, 9)          # [q, m, c, p, k]
    wt = np.ascontiguousarray(tern4.transpose(3, 2, 4, 0, 1)).astype(np.float16)

    # one packed [128, 268] f32 constant tensor -> a single DMA:
    #   cols 0:4     ind (group-average matrix, 1/32 one-hot)
    #   cols 4:132   indtg chunk0 (rows 0..3 used: gamma-weighted one-hot.T)
    #   cols 132:260 indtg chunk1 (rows 0..3 used)
    #   cols 260:262 sc chunk0 (scale, bias')   cols 262:264 sc chunk1
    #   cols 264:266 gb chunk0 (gamma, beta)    cols 266:268 gb chunk1
    g32 = gamma.astype(np.float32)
    pack = np.zeros((128, 268), np.float32)
    pack[np.arange(128), np.arange(128) // 32] = 1.0 / 32.0
    for cch in range(2):
        for p in range(128):
            pack[p // 32, 4 + 128 * cch + p] = g32[cch * 128 + p]
        pack[:, 260 + 2 * cch] = scale.reshape(2, 128)[cch]
        pack[:, 261 + 2 * cch] = bias2.reshape(2, 128)[cch]
        pack[:, 264 + 2 * cch] = g32.reshape(2, 128)[cch]
        pack[:, 265 + 2 * cch] = beta.astype(np.float32).reshape(2, 128)[cch]

    return dict(wt=wt, pack=pack, sqrt_s=sqrt_s)


def _build_nc(sqrt_s, ablate=None):
    import concourse.bacc as bacc
    import concourse.mybir as mybir
    import concourse.tile as tile
    from contextlib import ExitStack

    f32 = mybir.dt.float32
    f16 = mybir.dt.float16
    AF = mybir.ActivationFunctionType
    ALU = mybir.AluOpType

    # Bacc (not raw Bass): its compile() runs generate_event_semaphores,
    # which legalizes multi-sem waits down to the 1-wait-per-instruction
    # TRN2 ISA constraint.
    nc = bacc.Bacc("TRN2", target_bir_lowering=False, debug=False,
                   num_devices=N_CORES)

    x_ap = nc.dram_tensor("x", [B_LOC, C, HW], f32, kind="ExternalInput").ap()
    wt_ap = nc.dram_tensor("wt", [128, 2, 9, 2, 128], f16,
                           kind="ExternalInput").ap()
    pack_ap = nc.dram_tensor("pack", [128, 268], f32,
                             kind="ExternalInput").ap()
    out_ap = nc.dram_tensor("out", [B_LOC, C, HW], f32,
                            kind="ExternalOutput").ap()

    with tile.TileContext(nc) as tc, ExitStack() as ctx:
        consts = ctx.enter_context(tc.tile_pool(name="consts", bufs=1))
        x_pool = ctx.enter_context(tc.tile_pool(name="xp", bufs=2))
        u_pool = ctx.enter_context(tc.tile_pool(name="up", bufs=2))
        pad_pool = ctx.enter_context(tc.tile_pool(name="padp", bufs=1))
        st_pool = ctx.enter_context(tc.tile_pool(name="stp", bufs=2))
        tiny = ctx.enter_context(tc.tile_pool(name="tinyp", bufs=4))
        out_pool = ctx.enter_context(tc.tile_pool(name="outp", bufs=6))
        cps_pool = ctx.enter_context(tc.tile_pool(name="cps", bufs=5,
                                                  space="PSUM"))
        gps_pool = ctx.enter_context(tc.tile_pool(name="gps", bufs=1,
                                                  space="PSUM"))

        xs = [[None] * 2 for _ in range(B_LOC)]
        gms = [[None] * 2 for _ in range(B_LOC)]
        abs_ = [[None] * 2 for _ in range(B_LOC)]

        def emit_load_chunk(b, cch, bounds=(0, HW)):
            # split loads (at bn_stats 448-block boundaries) so stats on
            # early blocks start before the tail lands (Tile subtile deps)
            xt = x_pool.tile([128, HW], f32, name=f"x_{b}_{cch}",
                             tag=f"x{cch}")
            for lo, hi in zip(bounds[:-1], bounds[1:]):
                nc.sync.dma_start(
                    out=xt[:, lo:hi],
                    in_=x_ap[b, cch * 128:(cch + 1) * 128, lo:hi])
            xs[b][cch] = xt

        def emit_load(b, bounds=(0, HW)):
            for cch in range(2):
                emit_load_chunk(b, cch, bounds)

        # image 0 is on the critical path: chunk 0's x pieces dispatch first
        # (HWDGE dispatch is ~625ns per dma_start and serializes), then the
        # packed small consts, then chunk 1, then the conv weights
        emit_load_chunk(0, 0, bounds=(0, 3 * NN, 6 * NN, HW))
        pk_sb = consts.tile([128, 268], f32, name="pk_sb")
        nc.sync.dma_start(out=pk_sb, in_=pack_ap)
        emit_load_chunk(0, 1, bounds=(0, 3 * NN, 6 * NN, HW))

        w_sb = consts.tile([128, 2, 9, 2, 128], f16, name="w_sb")
        nc.sync.dma_start(out=w_sb, in_=wt_ap)

        ind_sb = pk_sb[:, 0:4]
        indtg_sb = [pk_sb[0:4, 4:132], pk_sb[0:4, 132:260]]
        sc_sb = [pk_sb[:, 260:262], pk_sb[:, 262:264]]
        gb_sb = [pk_sb[:, 264:266], pk_sb[:, 266:268]]
        eps_sb = consts.tile([4, 1], f32, name="eps_sb")
        nc.vector.memset(eps_sb, GN_EPS)

        pad_t = [[pad_pool.tile([128, PW, PW], f16, name=f"pad_{cch}_{s}",
                                tag=f"pad{cch}{s}")
                  for s in range(2)] for cch in range(2)]
        for cch in range(2):
            for s in range(2):
                # border-only memsets (interior is overwritten every image),
                # on GPSIMD which is otherwise idle — a full-tile DVE memset
                # costs 3.5us x4 and sits right in front of bn_stats
                pt = pad_t[cch][s]
                nc.gpsimd.memset(pt[:, 0, :], OFF)
                nc.gpsimd.memset(pt[:, PW - 1, :], OFF)
                nc.gpsimd.memset(pt[:, 1:PW - 1, 0:1], OFF)
                nc.gpsimd.memset(pt[:, 1:PW - 1, PW - 1:PW], OFF)

        def emit_stats(b, chunks=(0, 1)):
            for cch in chunks:
                st6 = st_pool.tile([128, RB, 6], f32, name=f"st6_{b}_{cch}",
                                   tag="st6")
                for j in range(RB):
                    nc.vector.bn_stats(out=st6[:, j, :],
                                       in_=xs[b][cch][:, j * NN:(j + 1) * NN])
                # rhs = (mean, var, mean^2) built by slice-writes (keeps
                # every matmul-operand producer on DVE: Matmult's LDWEIGHTS
                # slot only fits 2 sync waits, walrus NCC_INLA001)
                m3 = tiny.tile([128, 3], f32, name=f"m3_{b}_{cch}", tag="m3")
                nc.vector.bn_aggr(out=m3[:, 0:2], in_=st6)
                nc.vector.tensor_mul(m3[:, 2:3], m3[:, 0:1], m3[:, 0:1])
                gmv = gps_pool.tile([4, 3], f32, name=f"gmv_{b}_{cch}",
                                    tag="gmv")
                nc.tensor.matmul(gmv, lhsT=ind_sb, rhs=m3, start=True,
                                 stop=True)
                gm = tiny.tile([4, 3], f32, name=f"gm_{b}_{cch}", tag="gm")
                nc.vector.tensor_copy(out=gm, in_=gmv)
                gms[b][cch] = gm

        def emit_post(b, chunks=(0, 1)):
            for cch in chunks:
                gm = gms[b][cch]
                gsq = tiny.tile([4, 1], f32, name=f"gsq_{b}_{cch}", tag="gsq")
                nc.vector.tensor_mul(gsq, gm[:, 0:1], gm[:, 0:1])
                gvar = tiny.tile([4, 1], f32, name=f"gvar_{b}_{cch}",
                                 tag="gvar")
                # var_g = (avg var + avg mean^2) - mean_g^2, one fused op
                nc.vector.scalar_tensor_tensor(
                    out=gvar, in0=gm[:, 1:2], scalar=gm[:, 2:3], in1=gsq,
                    op0=ALU.add, op1=ALU.subtract)
                # gvar <- 1/sqrt(var + eps); reciprocal lands straight in
                # vals[:,1] to skip a copy on the critical chain
                nc.scalar.activation(out=gvar, in_=gvar, func=AF.Sqrt,
                                     bias=eps_sb, scale=1.0)
                vals = tiny.tile([4, 2], f32, name=f"vals_{b}_{cch}",
                                 tag="vals")
                nc.vector.reciprocal(out=vals[:, 1:2], in_=gvar)
                nc.vector.tensor_mul(vals[:, 0:1], gm[:, 0:1], vals[:, 1:2])
                bc = gps_pool.tile([128, 2], f32, name=f"bc_{b}_{cch}",
                                   tag="bc")
                nc.tensor.matmul(bc, lhsT=indtg_sb[cch], rhs=vals, start=True,
                                 stop=True)
                ab = tiny.tile([128, 2], f32, name=f"ab_{b}_{cch}", tag="ab")
                nc.vector.tensor_copy(out=ab, in_=bc)
                bt = tiny.tile([128, 1], f32, name=f"bt_{b}_{cch}", tag="bt")
                nc.vector.tensor_sub(bt, gb_sb[cch][:, 1:2], ab[:, 0:1])
                abs_[b][cch] = (ab, bt)

        def emit_act(b, split=1, chunks=(0, 1), dve_sq=False):
            # split>1 shortens chain latency (image 0 is on the critical
            # path before any conv work exists); dve_sq moves alternate
            # quarters' square onto DVE to unbottleneck ACT at startup
            nh = split
            hrows = H // nh
            hlen = hrows * W
            for cch in chunks:
                ab, bt = abs_[b][cch]
                u = u_pool.tile([128, HW], f32, name=f"u_{b}_{cch}",
                                tag=f"u{cch}")
                for h in range(nh):
                    sl = slice(h * hlen, (h + 1) * hlen)
                    uv = u[:, sl]
                    # u = relu(A*x + B)
                    nc.scalar.activation(out=uv, in_=xs[b][cch][:, sl],
                                         func=AF.Relu,
                                         bias=bt, scale=ab[:, 1:2])
                    if dve_sq and (h + cch) % 2 == 1:
                        # u = (u*u)*S on DVE (tensor_tensor_reduce)
                        tacc = tiny.tile([128, 1], f32,
                                         name=f"tacc_{b}_{cch}_{h}",
                                         tag="tacc")
                        nc.vector.tensor_tensor_reduce(
                            out=uv, in0=uv, in1=uv, scale=sqrt_s * sqrt_s,
                            scalar=0.0, op0=ALU.mult, op1=ALU.max,
                            accum_out=tacc)
                    else:
                        # u = (sqrt(S)*u)^2 = S*relu(z)^2
                        nc.scalar.activation(out=uv, in_=uv, func=AF.Square,
                                             scale=sqrt_s)
                    if ROUND_VIA_F16:
                        # pad interior = min(u,255) + 1024; the fp32->fp16
                        # output convert rounds to the integer grid (RNE;
                        # +1024 is even so tie parity matches jnp.round)
                        nc.vector.tensor_scalar(
                            out=pad_t[cch][b % 2][:, 1 + h * hrows:
                                                  1 + (h + 1) * hrows,
                                                  1:W + 1],
                            in0=uv.rearrange("p (h w) -> p h w", h=hrows),
                            scalar1=255.0, scalar2=OFF,
                            op0=ALU.min, op1=ALU.add)
                    else:
                        # u = min(u, 255) + C32 (fp32 add rounds to int, RNE)
                        nc.vector.tensor_scalar(out=uv, in0=uv, scalar1=255.0,
                                                scalar2=C32, op0=ALU.min,
                                                op1=ALU.add)
                        # pad interior = (u - C32) + 1024, exact f16 integers
                        nc.vector.tensor_scalar_add(
                            out=pad_t[cch][b % 2][:, 1 + h * hrows:
                                                  1 + (h + 1) * hrows,
                                                  1:W + 1],
                            in0=uv.rearrange("p (h w) -> p h w", h=hrows),
                            scalar1=OFF - C32)

        def emit_conv(b, q):
            # one full-image output tile per (b, q): evacs write slices and a
            # single big DMA stores it (HWDGE dispatch is ~625ns per
            # dma_start; 7 small stores would serialize against the loads)
            osb = out_pool.tile([128, HW], f32, name=f"o_{b}_{q}", tag="osb",
                                bufs=2)
            for rb in range(RB):
                ps = cps_pool.tile([128, NN], f32, name=f"ps_{b}_{q}_{rb}",
                                   tag="cps")
                i = 0
                for cch in range(2):  # cch-outer: taps of chunk 0 can start
                    for k in range(9):  # before chunk 1's pad is written
                        dy, dx = divmod(k, 3)
                        nc.tensor.matmul(
                            ps,
                            lhsT=w_sb[:, cch, k, q, :],
                            rhs=pad_t[cch][b % 2][:, rb * RBH + dy:
                                                  rb * RBH + dy + RBH,
                                                  dx:dx + W],
                            start=(i == 0), stop=(i == 17))
                        i += 1
                nc.scalar.activation(out=osb[:, rb * NN:(rb + 1) * NN],
                                     in_=ps, func=AF.Identity,
                                     bias=sc_sb[q][:, 1:2],
                                     scale=sc_sb[q][:, 0:1])
            # piecewise stores: each piece only depends on the evacs that
            # wrote it (subtile deps).  The last image streams out
            # per-rowblock so the kernel tail isn't gated on one big DMA.
            cuts = (list(range(0, HW + 1, NN)) if b == B_LOC - 1
                    else [0, 4 * NN, HW])
            for lo, hi in zip(cuts[:-1], cuts[1:]):
                nc.sync.dma_start(out=out_ap[b, q * 128:(q + 1) * 128, lo:hi],
                                  in_=osb[:, lo:hi])

        if ablate == "conv":
            for b in range(B_LOC):
                emit_conv(b, 0)
                emit_conv(b, 1)
        elif ablate == "gn":
            emit_stats(0); emit_post(0); emit_act(0)
            for b in range(1, B_LOC):
                emit_load(b); emit_stats(b); emit_post(b); emit_act(b)
        else:
            for cch in range(2):
                emit_stats(0, chunks=(cch,))
                emit_post(0, chunks=(cch,))
                emit_act(0, split=4, chunks=(cch,))
            for b in range(B_LOC):
                if b + 1 < B_LOC:
                    emit_load(b + 1)
                    emit_stats(b + 1)
                emit_conv(b, 0)
                if b + 1 < B_LOC:
                    emit_post(b + 1)
                emit_conv(b, 1)
                if b + 1 < B_LOC:
                    emit_act(b + 1)

    nc.compile()
    return nc


def kernel(x, gamma, beta, a, weight_fp, bias):
    consts = _host_prep(np.asarray(gamma), np.asarray(beta), np.asarray(a),
                        np.asarray(weight_fp), np.asarray(bias))
    nc = _build_nc(consts.pop("sqrt_s"))

    from concourse.bass_utils import run_bass_kernel_spmd

    x = np.ascontiguousarray(np.asarray(x, dtype=np.float32)
                             .reshape(B_TOT, C, HW))
    in_maps = []
    for core in range(N_CORES):
        in_maps.append({
            "x": x[core * B_LOC:(core + 1) * B_LOC],
            "wt": consts["wt"],
            "pack": consts["pack"],
        })
    res = run_bass_kernel_spmd(nc, in_maps, list(range(N_CORES)))
    out = np.concatenate([res.results[i]["out"] for i in range(N_CORES)],
                         axis=0)
    return out.reshape(B_TOT, C, H, W)


if __name__ == "__main__":
    rng = np.random.default_rng(0)
    x = rng.standard_normal((B_TOT, C, H, W), dtype=np.float32)
    out = kernel(x, np.ones(C, np.float32), np.zeros(C, np.float32),
                 np.float32(6.0),
                 rng.standard_normal((C, C, 3, 3), dtype=np.float32) * 0.03,
                 np.zeros(C, np.float32))
    print(out.shape, out.dtype)
